# revision 15
# baseline (speedup 1.0000x reference)
"""CycleVAR VQ-codebook encoder kernel for Trainium2 (8 NeuronCores).

Contract: kernel(**inputs) takes FULL inputs
  f_src      [128, 32, 16, 16] fp32
  emb_weight [4096, 32] fp32
and returns the FULL output x_var [128, 340, 32] fp32.

Observation: the reference's x_var depends only on quantization stages
pn in (1, 2, 4, 8); the pn=16 stage's outputs are never used. So only
85 tokens/image are quantized.

Sharding: data-parallel over batch (16 images per core), codebook and
resize matrices replicated. No cross-core communication.

v2 design notes (per-core, B=16, C=32, H=W=16, S=256):
  - PE warmup matmuls on a zero tile at program start so the HAM clock
    gate lifts 1.2->2.4 GHz before real work.
  - f layout "spatial": [s(part, 2x128), (b,c)(free, 512)]
  - z-down batched in 4-image groups: z4_j[(b'c), p] via 2 accumulating
    matmuls over s-chunks; 16 small ACT copies assemble zaug [33, ntok].
  - scores: eaug columns HOST-PERMUTED so that reduce-group id gg in
    [0,32) and in-group index j in [0,128) give code id = 128*gg + j
    directly.  Per 128-token block: 2 psum tiles [128,2048] (4 banks
    each), 4 fp32 matmuls per tile 2x-packed on PE row-groups {0,1} and
    {2,3} (zaug/eaug replicated at partition 64).
  - argmax: DVE grouped reduce_max per psum tile (g=16) -> gmax[tw,32];
    max8 + max_index -> winning group gg*; scores roundtrip to DRAM
    (ACT copy + DMA export per tile); indirect-DMA gathers row
    (t*32+gg*) -> max_index -> j*; vidx = 128*gg* + j*.
  - h: per-image indirect emb gather straight into h_sp [p, (b,c)]
    (stage 0 falls back to a DRAM-staging rearrange).
  - stage tails split by (b,c)-halves: up-matmul, f_rest/f_partial
    updates and the NEXT stage's z-down run per 8-image half so the
    second half overlaps the first half's argmax chain.
Output DRAM x_out [340, (b,c)]; host transposes to [b, 340, c].
"""

import os

import numpy as np

import concourse.bacc as bacc
import concourse.bass as bass
import concourse.mybir as mybir
import concourse.tile as tile
from concourse.bass import IndirectOffsetOnAxis
from concourse.bass_utils import run_bass_kernel_spmd

N_CORES = 8
B_FULL = 128
B_LOC = B_FULL // N_CORES  # 16
C = 32
H = 16
S = H * H  # 256
V = 4096
PNS = (1, 2, 4, 8)
ROW_OFF = (0, 4, 20, 84)  # x_var row offsets per stage
NTOK_OUT = 340

F32 = mybir.dt.float32
U32 = mybir.dt.uint32
AX = mybir.AxisListType
ALU = mybir.AluOpType
ACTF = mybir.ActivationFunctionType

LAST_RESULTS = None  # test harness introspection


def _keys_cubic(x, a=-0.5):
    x = np.abs(x)
    return np.where(
        x <= 1,
        (a + 2) * x**3 - (a + 3) * x**2 + 1,
        np.where(x < 2, a * x**3 - 5 * a * x**2 + 8 * a * x - 4 * a, 0.0),
    )


def _resize_matrix_1d(n_in, n_out):
    # matches jax.image.resize(method='cubic') for upsampling
    scale = n_out / n_in
    U = np.zeros((n_out, n_in), np.float64)
    for i in range(n_out):
        x = (i + 0.5) / scale - 0.5
        w = _keys_cubic(x - np.arange(n_in))
        s = w.sum()
        if s != 0:
            w = w / s
        U[i] = w
    return U


def _up_matrix(pn):
    # [S, pn*pn] bicubic upsample matrix (kron of separable 1D)
    if pn == H:
        return np.eye(S, dtype=np.float32)
    U1 = _resize_matrix_1d(pn, H)
    return np.kron(U1, U1).astype(np.float32)


def _down_matrix(pn):
    # [pn*pn, S] exact area mean (r = H//pn, weight 1/r^2, exact pow2)
    r = H // pn
    A = np.zeros((pn * pn, S), np.float32)
    w = np.float32(1.0 / (r * r))
    for pi in range(pn):
        for pj in range(pn):
            for di in range(r):
                for dj in range(r):
                    A[pi * pn + pj, (pi * r + di) * H + (pj * r + dj)] = w
    return A


def _col_perm():
    # matmul column x = 2048*h + 512*q + 128*s + i holds original code
    # pid = 128*(8*q + 4*h + s) + i, so that the grouped-max group id gg
    # (gmax column) and in-group offset j recover the code as 128*gg+j.
    x = np.arange(V)
    h = x >> 11
    q = (x >> 9) & 3
    s = (x >> 7) & 3
    i = x & 127
    return 128 * (8 * q + 4 * h + s) + i  # pid[x]


V_WARM = os.environ.get("CVAR_NOWARM", "") == ""  # PE warmup matmuls
V_BIGPSUM = os.environ.get("CVAR_PSUM1024", "") == ""  # [128,2048] score tiles
V_GROUPZ = os.environ.get("CVAR_SIMPLEZ", "") == ""  # grouped z-down + strided DMA
V_IMGGATHER = os.environ.get("CVAR_NOIMG", "") == ""  # per-image emb gather


def _build_program():
    nc = bacc.Bacc(trn_type="TRN2", target_bir_lowering=False, debug=False)

    # DRAM I/O (per core)
    f_in = nc.dram_tensor("f_pre", [2, 128, 512], F32, kind="ExternalInput").ap()
    eaug_in = nc.dram_tensor("eaug", [33, V], F32, kind="ExternalInput").ap()
    emb_in = nc.dram_tensor("embt", [V, C], F32, kind="ExternalInput").ap()
    a_in = {
        pn: nc.dram_tensor(f"a{pn}", [2, 128, pn * pn], F32, kind="ExternalInput").ap()
        for pn in PNS
    }
    u_in = {
        pn: nc.dram_tensor(f"u{pn}", [pn * pn, 256], F32, kind="ExternalInput").ap()
        for pn in PNS
    }
    x_out = nc.dram_tensor("xout", [NTOK_OUT, 512], F32, kind="ExternalOutput").ap()

    with tile.TileContext(nc) as tc:
        from contextlib import ExitStack

        ctx = ExitStack()
        const = ctx.enter_context(tc.tile_pool(name="const", bufs=1))
        work = ctx.enter_context(tc.tile_pool(name="work", bufs=2))
        small = ctx.enter_context(tc.tile_pool(name="small", bufs=3))
        psum = ctx.enter_context(tc.tile_pool(name="psum", bufs=2, space="PSUM"))
        dram = ctx.enter_context(tc.tile_pool(name="dram", bufs=2, space="DRAM"))

        # ---- PE warmup: matmuls on a zeroed tile, no DMA dependency,
        # ~4us of activity flips the HAM clock gate to 2.4 GHz ----
        wz = const.tile([33, 128], F32, name="warmz")
        nc.vector.memset(wz, 0.0)
        if V_WARM:
            pw = psum.tile([64, 128], F32, tag="psq")
            for _ in range(8):
                nc.tensor.matmul(pw, wz[:, 0:64], wz, start=True, stop=True)

        # ---- constants to SBUF ----
        f_rest = [const.tile([128, 512], F32, name=f"frest{ch}") for ch in range(2)]
        f_partial = [const.tile([128, 512], F32, name=f"fpart{ch}") for ch in range(2)]
        a_sb = {}
        for pn in PNS:
            P = pn * pn
            a_sb[pn] = [const.tile([128, P], F32, name=f"a{pn}_{ch}") for ch in range(2)]
        for ch in range(2):
            nc.sync.dma_start(f_rest[ch], f_in[ch])
        for ch in range(2):
            nc.scalar.dma_start(a_sb[1][ch], a_in[1][ch])
        eaug_big = const.tile([97, V], F32)
        nc.sync.dma_start(eaug_big[0:33], eaug_in)
        nc.scalar.dma_start(eaug_big[64:97], eaug_in)
        for pn in PNS[1:]:
            for ch in range(2):
                nc.sync.dma_start(a_sb[pn][ch], a_in[pn][ch])
        u_sb = {}
        for pn in PNS:
            P = pn * pn
            u_sb[pn] = const.tile([128, 256], F32, name=f"u{pn}")
            nc.scalar.dma_start(u_sb[pn][0:P], u_in[pn])
            nc.sync.dma_start(u_sb[pn][64 : 64 + P], u_in[pn])
        for ch in range(2):
            nc.vector.memset(f_partial[ch], 0.0)

        zaug_big = const.tile([97, 1024], F32)
        zaug = zaug_big[0:33]
        nc.vector.memset(zaug_big[32:33, :], 1.0)
        nc.vector.memset(zaug_big[96:97, :], 1.0)

        toff32 = const.tile([128, 1], U32)  # t*32 per partition
        nc.gpsimd.iota(toff32, pattern=[[1, 1]], base=0, channel_multiplier=32)

        hstage = dram.tile([128, C], F32, tag="hstage", bufs=1)

        def z_phase(pn, hf):
            """z-down for images [8*hf, 8*hf+8): two 4-image grouped
            matmuls -> ACT bulk copy to SBUF -> partition-shift DMAs
            into zaug token layout (rows 0:32 and the 64-replica)."""
            P = pn * pn
            if not V_GROUPZ:
                # baseline-style per-image z-down
                for b in range(8 * hf, 8 * hf + 8):
                    psz = psum.tile([32, max(P, 1)], F32, tag="psz")
                    for ch in range(2):
                        nc.tensor.matmul(
                            psz[:, :P],
                            f_rest[ch][:, 32 * b : 32 * (b + 1)],
                            a_sb[pn][ch][:, :P],
                            start=(ch == 0),
                            stop=(ch == 1),
                        )
                    nc.scalar.activation(
                        zaug[0:32, b * P : (b + 1) * P], psz[:, :P], ACTF.Copy
                    )
                cs = slice(8 * hf * P, 8 * (hf + 1) * P)
                q = nc.scalar if hf == 0 else nc.sync
                q.dma_start(zaug_big[64:96, cs], zaug_big[0:32, cs])
                return
            psz = psum.tile([128, max(2 * P, 8)], F32, tag="psq")
            for j2, j in enumerate((2 * hf, 2 * hf + 1)):
                for ch in range(2):
                    nc.tensor.matmul(
                        psz[:, j2 * P : (j2 + 1) * P],
                        f_rest[ch][:, 128 * j : 128 * (j + 1)],
                        a_sb[pn][ch],
                        start=(ch == 0),
                        stop=(ch == 1),
                    )
            z4sb = work.tile([128, max(2 * P, 8)], F32, tag="z4sb")
            nc.scalar.activation(z4sb[:, : 2 * P], psz[:, : 2 * P], ACTF.Copy)
            # zaug cols for image b = 8*hf + 4*j2 + b2 start at b*P; view the
            # half's region as [c, j2, b2, p] so each b2's DMA is one 3-D AP.
            qs = [nc.scalar, nc.sync]
            for base in (0, 64):
                dst = zaug_big[base : base + 32, 8 * hf * P : 8 * (hf + 1) * P]
                dst = dst.rearrange("c (j2 b2 p) -> c j2 b2 p", j2=2, b2=4)
                for b2 in range(4):
                    src = z4sb[32 * b2 : 32 * b2 + 32, : 2 * P].rearrange(
                        "c (j2 p) -> c j2 p", j2=2
                    )
                    qs[(b2 + base // 64) % 2].dma_start(dst[:, :, b2], src)

        # stage 0 z for both halves
        for hf in range(2):
            z_phase(1, hf)

        for si, pn in enumerate(PNS):
            P = pn * pn
            ntok = B_LOC * P
            nblk = (ntok + 127) // 128

            h_sp = work.tile([128, 512], F32, tag="hsp", name=f"hsp{si}")
            scd = dram.tile([4096, 128], F32, tag="scd")

            # ---- blocks: scores + argmax + h gather ----
            for blk in range(nblk):
                t0 = blk * 128
                tw = min(128, ntok - t0)
                b0 = t0 // P
                nb = tw // P  # images in this block (1 for small stages)

                gmax = small.tile([128, 32], F32, tag="gmax")
                scsb = work.tile([128, V], F32, tag="scsb")
                W = 2048 if V_BIGPSUM else 1024
                nT = V // W
                gpt = W // 128  # reduce groups per tile
                for T in range(nT):
                    pq = psum.tile([128, W], F32, tag="psq")
                    for t2 in range(W // 1024):
                        qc = (W // 1024) * T + t2
                        c0 = 1024 * t2
                        nc.tensor.matmul(
                            pq[:tw, c0 : c0 + 512],
                            zaug_big[0:33, t0 : t0 + tw],
                            eaug_big[0:33, 512 * qc : 512 * (qc + 1)],
                            start=True,
                            stop=True,
                            tile_position=(0, 0),
                        )
                        nc.tensor.matmul(
                            pq[:tw, c0 + 512 : c0 + 1024],
                            zaug_big[64:97, t0 : t0 + tw],
                            eaug_big[64:97, 2048 + 512 * qc : 2048 + 512 * (qc + 1)],
                            start=True,
                            stop=True,
                            tile_position=(64, 0),
                        )
                    nc.vector.tensor_reduce(
                        gmax[:tw, gpt * T : gpt * (T + 1)],
                        pq[:tw].rearrange("t (g s) -> t g s", g=gpt),
                        axis=AX.X,
                        op=ALU.max,
                    )
                    nc.scalar.activation(
                        scsb[:tw, W * T : W * (T + 1)], pq[:tw], ACTF.Copy
                    )
                    nc.sync.dma_start(
                        scd.rearrange("(t g) s -> t g s", g=32)[
                            :tw, gpt * T : gpt * (T + 1)
                        ],
                        scsb[:tw, W * T : W * (T + 1)].rearrange(
                            "t (g s) -> t g s", g=gpt
                        ),
                    )

                top8 = small.tile([128, 8], F32, tag="top8")
                gg8 = small.tile([128, 8], U32, tag="gg8")
                nc.vector.max(top8[:tw], gmax[:tw])
                nc.vector.max_index(gg8[:tw], top8[:tw], gmax[:tw])

                off = small.tile([128, 1], U32, tag="off")
                nc.vector.tensor_tensor(
                    off[:tw], toff32[:tw], gg8[:tw, 0:1], op=ALU.add
                )
                grp = small.tile([128, 128], F32, tag="grp")
                nc.gpsimd.indirect_dma_start(
                    grp[:tw], None, scd[:, :], IndirectOffsetOnAxis(ap=off[:tw], axis=0)
                )
                j8 = small.tile([128, 8], U32, tag="j8")
                nc.vector.max_index(j8[:tw], top8[:tw], grp[:tw])
                vidx = small.tile([128, 1], U32, tag="vidx")
                nc.vector.tensor_scalar_mul(vidx[:tw], gg8[:tw, 0:1], 128)
                nc.vector.tensor_tensor(
                    vidx[:tw], vidx[:tw], j8[:tw, 0:1], op=ALU.add
                )

                if P >= 16 and V_IMGGATHER:
                    # gather emb rows straight into h_sp [p, (b,c)]
                    for b2 in range(nb):
                        b = b0 + b2
                        nc.gpsimd.indirect_dma_start(
                            h_sp[0:P, 32 * b : 32 * (b + 1)],
                            None,
                            emb_in,
                            IndirectOffsetOnAxis(
                                ap=vidx[b2 * P : (b2 + 1) * P], axis=0
                            ),
                        )
                else:
                    # one gather + DRAM-staging rearrange
                    htok = small.tile([128, C], F32, tag="htok")
                    nc.gpsimd.indirect_dma_start(
                        htok[:tw],
                        None,
                        emb_in,
                        IndirectOffsetOnAxis(ap=vidx[:tw], axis=0),
                    )
                    nc.scalar.dma_start(hstage[:tw], htok[:tw])
                    nc.sync.dma_start(
                        h_sp[0:P, 32 * b0 : 32 * (b0 + nb)].rearrange(
                            "p (b c) -> p b c", c=C
                        ),
                        hstage[:tw].rearrange("(b p) c -> p b c", p=P),
                    )

            # ---- per-half: up + updates + next-stage z ----
            for hf in range(2):
                cs = slice(256 * hf, 256 * (hf + 1))
                qrep = nc.scalar if hf == 0 else nc.sync
                qrep.dma_start(h_sp[64 : 64 + P, cs], h_sp[0:P, cs])
                pus = []
                for ch in range(2):
                    pu = psum.tile([128, 256], F32, tag="psq")
                    nc.tensor.matmul(
                        pu,
                        u_sb[pn][64 * ch : 64 * ch + P, 128 * ch : 128 * (ch + 1)],
                        h_sp[64 * ch : 64 * ch + P, cs],
                        start=True,
                        stop=True,
                        tile_position=(64 * ch, 0),
                    )
                    pus.append(pu)
                for ch in range(2):
                    pu = pus[ch]
                    if si < 3:
                        nc.vector.tensor_tensor(
                            f_rest[ch][:, cs], f_rest[ch][:, cs], pu, op=ALU.subtract
                        )
                    nc.vector.tensor_tensor(
                        f_partial[ch][:, cs], f_partial[ch][:, cs], pu, op=ALU.add
                    )
                if si < 3:
                    z_phase(PNS[si + 1], hf)

            # ---- x output ----
            if si < 3:
                pn2 = PNS[si + 1]
                P2 = pn2 * pn2
                px = psum.tile([128, 512], F32, tag="psq")
                for ch in range(2):
                    nc.tensor.matmul(
                        px[:P2],
                        a_sb[pn2][ch][:, :P2],
                        f_partial[ch],
                        start=(ch == 0),
                        stop=(ch == 1),
                    )
                x_sb = small.tile([max(P2, 1), 512], F32, tag="xsb")
                nc.scalar.activation(x_sb[:P2], px[:P2], ACTF.Copy)
                nc.scalar.dma_start(x_out[ROW_OFF[si] : ROW_OFF[si] + P2], x_sb[:P2])
            else:
                for ch in range(2):
                    for hf in range(2):
                        cs = slice(256 * hf, 256 * (hf + 1))
                        qs2 = [nc.sync, nc.scalar]
                        qs2[hf].dma_start(
                            x_out[84 + 128 * ch : 84 + 128 * (ch + 1), cs],
                            f_partial[ch][:, cs],
                        )

        ctx.close()

    nc.compile()
    return nc


_PROGRAM = None


def _get_program():
    global _PROGRAM
    if _PROGRAM is None:
        _PROGRAM = _build_program()
    return _PROGRAM


def kernel(f_src, emb_weight):
    global LAST_RESULTS
    f_src = np.asarray(f_src, dtype=np.float32)
    emb_weight = np.asarray(emb_weight, dtype=np.float32)

    e64 = emb_weight.astype(np.float64)
    eaug_base = np.concatenate(
        [emb_weight.T, (-0.5 * (e64 * e64).sum(1)).astype(np.float32)[None, :]], axis=0
    )  # [33, V]
    eaug = np.ascontiguousarray(eaug_base[:, _col_perm()])

    a_mats = {}
    u_mats = {}
    for pn in PNS:
        P = pn * pn
        a_mats[pn] = np.ascontiguousarray(
            _down_matrix(pn).T.reshape(2, 128, P)
        )  # [2, 128, P]
        u_mats[pn] = np.ascontiguousarray(_up_matrix(pn).T)  # [P, 256]

    in_maps = []
    for core in range(N_CORES):
        fb = f_src[core * B_LOC : (core + 1) * B_LOC]  # [16, 32, 16, 16]
        f_pre = (
            fb.reshape(B_LOC, C, S).transpose(2, 0, 1).reshape(2, 128, 512)
        )  # [s, b, c]
        m = {
            "f_pre": np.ascontiguousarray(f_pre),
            "eaug": eaug,
            "embt": np.ascontiguousarray(emb_weight),
        }
        for pn in PNS:
            m[f"a{pn}"] = a_mats[pn]
            m[f"u{pn}"] = u_mats[pn]
        in_maps.append(m)

    nc = _get_program()
    trace = bool(os.environ.get("CVAR_TRACE"))
    try:
        res = run_bass_kernel_spmd(
            nc,
            in_maps,
            core_ids=list(range(N_CORES)),
            trace=trace,
        )
    except ModuleNotFoundError:
        res = run_bass_kernel_spmd(
            nc, in_maps, core_ids=list(range(N_CORES)), trace=False
        )
    LAST_RESULTS = res

    outs = []
    for core in range(N_CORES):
        xo = res.results[core]["xout"]  # [340, 512]
        outs.append(xo.reshape(NTOK_OUT, B_LOC, C).transpose(1, 0, 2))
    return np.ascontiguousarray(np.concatenate(outs, axis=0))
